# revision 1
# baseline (speedup 1.0000x reference)
"""Trainium2 Bass kernel for nn_EulerFullAttention.

Math (per batch b, head h, dh=64):
  theta_q = x/(1+|w_q|) + b_q + t*phi_q ; Q = [cos(theta_q), sin(theta_q)]  (S,128)
  theta_k likewise ; K = [cos, sin]
  V = cos(theta_v)+sin(theta_v) = sqrt(2)*sin(theta_v + pi/4)              (S,64)
  scores = Q @ K^T / sqrt(128), causal softmax, out = attn @ V
  result = sqrt(2)*sin(theta_o + pi/4), theta_o = out/(1+|w_out|) + b_out

Distribution: 8 cores = 2 batches x 4 head-groups (4 heads each). Each core
computes its x[:, 256-col] slice end to end; no collectives.

Trig via range reduction: r = theta/(2*pi) + c ; i = int32(r) (rounds to
nearest) ; f = r - i in [-0.5, 0.5] ; sin(theta) = Sin(2*pi*f) on the ACT
engine.  cos rows add +0.25 to c (in the host-precomputed t*phi2+c2 table);
the +pi/4 of V / the output layer folds +0.125 into ACT bias constants.

Layout: attention runs transposed, scoresT[k, q] = KT.T @ QT with QT/KT
feature-major fp16 [128, S] (rows 0:64 cos / 64:128 sin; PE-transposed x is
duplicated into both halves).  Scores are packed densely (cross-q-chunk)
into [128,1024] 2-bank PSUM tiles -> 17 wide exp instructions per head,
fp16 outputs.  Causal boundary zeroed by gpsimd affine_select.  attn@V
accumulates ot[65, 512] per 512-wide q chunk with lhsT = [V | 1] fp16 so
row 64 gives the softmax denominator; a PE transpose returns to natural
layout and one scalar_tensor_tensor fuses 1/rowsum with the per-feature
output scale.

Schedule (engines execute their queues in program order, so emission order
IS the schedule):
  - per-head phases: ACT does [sins head j] then [exps head j] while DVE /
    Pool / PE run head j+1's range-reduction pipeline and head j's
    attn@V+renorm; sync deps pin the ACT table grouping (10 table loads).
  - manually pre-placed InstLoadActFuncSet hides exp-table loads behind the
    first score matmuls of each phase.
  - attn@V of q-chunk qc is emitted after the scores of qc+1 so the in-order
    PE queue never blocks the next exp's input on exp-dependent work.
  - head 0's PSUM->SBUF copies (and the cos/sin duplication) run on the
    otherwise-idle ACT engine during the lead-in; x DMA is split so early
    chunks unlock the pipeline sooner; the last output quarter's range
    reduction runs on DVE (tail path) instead of Pool.
"""

import sys, math

sys.path.insert(0, "/opt/trn_rl_repo")

import numpy as np
import concourse.bass as bass
import concourse.mybir as mybir
from concourse.bacc import Bacc
from concourse.tile import TileContext
from concourse.bass_utils import run_bass_kernel_spmd
from contextlib import ExitStack

F32 = mybir.dt.float32
F16 = mybir.dt.float16
I32 = mybir.dt.int32
AF = mybir.ActivationFunctionType
ALU = mybir.AluOpType

B, S, D, H = 2, 2048, 1024, 16
DH = 64
NH = 4            # heads per core
DC = NH * DH      # 256 feature columns per core
NB = S // 128     # 16 s-blocks
TWO_PI = 2.0 * math.pi
SQRT2 = math.sqrt(2.0)
EXP_SCALE = 1.0 / math.sqrt(2.0 * DH)


def _bcast_mid(ap2d, n):
    """[128, F] AP -> [128, n, F] with stride-0 middle dim."""
    return bass.AP(tensor=ap2d.tensor, offset=ap2d.offset,
                   ap=[ap2d.ap[0], [0, n], ap2d.ap[-1]])


PACKW = 1024

def _build_head_packs():
    """Score-strip layout for a whole head, packed densely into [128,PACKW]
    PSUM tiles across q-chunk boundaries.  Returns (packs, qc_strips):
    packs = [(width, [(qc, kb, qs, N, off)])]; qc_strips[qc] = list of
    (pack_idx, kb, qs, N, off).  Strips never cross a 512 PSUM bank
    boundary and every pack is written contiguously from offset 0."""
    strips = []
    for qc in range(4):
        for kb in range(4 * qc):
            strips.append((qc, kb, 512 * qc, 512))
        for jj in (0, 1, 3, 2):   # 512, 384, 128, 256: packs stay gap-free
            kb = 4 * qc + jj
            strips.append((qc, kb, 512 * qc + 128 * jj, 512 - 128 * jj))
    packs, cur, off = [], [], 0
    qc_strips = [[] for _ in range(4)]
    for (qc, kb, qs, N) in strips:
        o = off
        if o % 512 + N > 512:
            o = (o // 512 + 1) * 512
        if o + N > PACKW:
            packs.append((off, cur))
            cur, o = [], 0
        cur.append((qc, kb, qs, N, o))
        qc_strips[qc].append((len(packs), kb, qs, N, o))
        off = o + N
    if cur:
        packs.append((off, cur))
    return packs, qc_strips


def build_nc(tphi_sig=(0,) * 8, c_v=0.125, c_o=0.125):
    """tphi_sig[j*2+pi] = group id of the (s*phi2+c2) table for head j, proj
    pi; equal ids share one table. Tables come precomputed from DRAM when few
    groups; otherwise built on-chip from an iota."""
    ngroups = len(set(tphi_sig))
    nc = Bacc(trn_type="TRN2")
    xin = nc.dram_tensor("xin", [S, DC], F32, kind="ExternalInput")
    pars_d = nc.dram_tensor("pars", [128, NH * 6 + 2 * DC + 128], F32,
                            kind="ExternalInput")
    out_d = nc.dram_tensor("out", [S, DC], F32, kind="ExternalOutput")

    with TileContext(nc) as tc, ExitStack() as ctx:
        sing = ctx.enter_context(tc.tile_pool(name="sing", bufs=1))
        r2p = ctx.enter_context(tc.tile_pool(name="r2p", bufs=2))
        x2tp = ctx.enter_context(tc.tile_pool(name="x2tp", bufs=2))
        i2p = ctx.enter_context(tc.tile_pool(name="i2p", bufs=1))
        midi = ctx.enter_context(tc.tile_pool(name="midi", bufs=2))
        mid = ctx.enter_context(tc.tile_pool(name="mid", bufs=4))
        expool = ctx.enter_context(tc.tile_pool(name="exp", bufs=11))
        otpool = ctx.enter_context(tc.tile_pool(name="otp", bufs=3))
        tiny = ctx.enter_context(tc.tile_pool(name="tiny", bufs=4))
        tphip = ctx.enter_context(
            tc.tile_pool(name="tphip", bufs=min(ngroups, 2)))
        psp = ctx.enter_context(tc.tile_pool(name="psp", bufs=2, space="PSUM"))
        pso = ctx.enter_context(tc.tile_pool(name="pso", bufs=2, space="PSUM"))
        psn = ctx.enter_context(tc.tile_pool(name="psn", bufs=1, space="PSUM"))
        psx = ctx.enter_context(tc.tile_pool(name="psx", bufs=1, space="PSUM"))

        pars = sing.tile([128, NH * 6 + 2 * DC + 128], F32)
        nc.sync.dma_start(out=pars, in_=pars_d[:, :])

        def qkpc(j, c):
            return pars[:, 6 * j + c:6 * j + c + 1]

        vp = pars[:, NH * 6:NH * 6 + DC]
        opr = pars[:, NH * 6 + DC:NH * 6 + 2 * DC]
        identr = pars[:, NH * 6 + 2 * DC:NH * 6 + 2 * DC + 128]
        iota = sing.tile([128, S], I32)
        nc.gpsimd.iota(iota, pattern=[[1, S]], base=0, channel_multiplier=0)
        x_s = sing.tile([128, NB, DC], F32)
        xin_r = xin[:, :].rearrange("(n p) d -> p n d", p=128)
        nc.sync.dma_start(out=x_s[:, 0:2, :], in_=xin_r[:, 0:2, :])
        nc.sync.dma_start(out=x_s[:, 2:4, :], in_=xin_r[:, 2:4, :])
        bz = sing.tile([128, 1], F32)
        nc.vector.memset(bz, 0.0)
        bcv = sing.tile([128, 1], F32)
        nc.vector.memset(bcv, TWO_PI * c_v)
        bco = sing.tile([128, 1], F32)
        nc.vector.memset(bco, TWO_PI * c_o)
        vaug = []
        for j in range(NH):
            t = sing.tile([128, NB, DH + 1], F16, tag=f"vaug{j}")
            nc.vector.memset(t[:, :, DH:DH + 1], 1.0)
            vaug.append(t)
        ro_pre = sing.tile([128, NB, DC], F32)
        QKT = [sing.tile([128, 2, S], F16, tag=f"qkt{j}", name=f"qkt{j}")
               for j in range(NH)]

        tphi_tiles = {}

        def get_tphi(j, pi):
            g = tphi_sig[2 * j + pi]
            if g not in tphi_tiles:
                tph = tphip.tile([128, S], F32, tag="tphi")
                c0 = 3 * pi
                nc.vector.tensor_scalar(out=tph, in0=iota,
                                        scalar1=qkpc(j, c0 + 1),
                                        scalar2=qkpc(j, c0 + 2),
                                        op0=ALU.mult, op1=ALU.add)
                tphi_tiles[g] = tph
            return tphi_tiles[g]

        get_tphi(0, 0)
        get_tphi(0, 1)
        for qq in range(1, 4):
            nc.sync.dma_start(out=x_s[:, 4 * qq:4 * qq + 4, :],
                              in_=xin_r[:, 4 * qq:4 * qq + 4, :])

        sin_insts = []

        def v_quarter(qq):
            rv = mid.tile([128, 4, DC], F32, tag="mid")
            nc.vector.tensor_tensor(out=rv, in0=x_s[:, 4 * qq:4 * qq + 4, :],
                                    in1=_bcast_mid(vp[:, :], 4), op=ALU.mult)
            iv = midi.tile([128, 4, DC], I32, tag="midi")
            nc.vector.tensor_scalar(out=iv, in0=rv, scalar1=c_v, scalar2=None,
                                    op0=ALU.add)
            nc.gpsimd.tensor_tensor(out=rv, in0=rv, in1=iv, op=ALU.subtract)
            sv = mid.tile([128, 4, DC], F16, tag="mid16")
            si = nc.scalar.activation(out=sv, in_=rv, func=AF.Sin,
                                      bias=bcv[:, 0:1], scale=TWO_PI)
            sin_insts.append(si)
            for j in range(NH):
                nc.vector.tensor_copy(out=vaug[j][:, 4 * qq:4 * qq + 4, 0:DH],
                                      in_=sv[:, :, DH * j:DH * j + DH])

        prep_state = {}

        def prep_cc(j, cc):
            """One 512-column chunk of head j's q/k range-reduction chain."""
            if cc == 0:
                prep_state[j] = (r2p.tile([128, 2, S], F32, tag="r2", name=f"r2_{j}"),
                                 i2p.tile([128, 2, S], I32, tag="i2", name=f"i2_{j}"))
            r2, i2 = prep_state[j]
            xtp = psx.tile([64, 512], F32, tag="px")
            for sb in range(4):
                n = 4 * cc + sb
                nc.tensor.transpose(xtp[:, 128 * sb:128 * sb + 128],
                                    x_s[:, n, DH * j:DH * j + DH], identr)
            x2t = x2tp.tile([128, 512], F32, tag="x2t")
            if j == 0:   # ACT is idle during the lead-in; offload the copies
                nc.scalar.copy(out=x2t[0:64, :], in_=xtp)
                nc.scalar.copy(out=x2t[64:128, :], in_=xtp)
            else:
                nc.vector.tensor_copy(out=x2t[0:64, :], in_=xtp)
                nc.sync.dma_start(out=x2t[64:128, :], in_=x2t[0:64, :])
            sl = slice(512 * cc, 512 * cc + 512)
            nc.vector.scalar_tensor_tensor(
                out=r2[:, 0, sl], in0=x2t, scalar=qkpc(j, 0),
                in1=get_tphi(j, 0)[:, sl], op0=ALU.mult, op1=ALU.add)
            nc.vector.scalar_tensor_tensor(
                out=r2[:, 1, sl], in0=x2t, scalar=qkpc(j, 3),
                in1=get_tphi(j, 1)[:, sl], op0=ALU.mult, op1=ALU.add)
            nc.vector.tensor_copy(out=i2[:, :, sl], in_=r2[:, :, sl])
            nc.vector.scalar_tensor_tensor(
                out=r2[:, 0, sl], in0=i2[:, 0, sl], scalar=-1.0,
                in1=r2[:, 0, sl], op0=ALU.mult, op1=ALU.add)
            nc.gpsimd.tensor_tensor(out=r2[:, 1, sl], in0=r2[:, 1, sl],
                                    in1=i2[:, 1, sl], op=ALU.subtract)

        def qk_sin(j):
            r2, _ = prep_state[j]
            si = nc.scalar.activation(out=QKT[j], in_=r2, func=AF.Sin,
                                      bias=bz[:, 0:1], scale=TWO_PI)
            sin_insts.append(si)
            return si

        EXP_SET, SIN_SET = 0, 9   # act_info.json ids: exp_and_others, trig_and_small

        def preload_table(set_id):
            ins = mybir.InstLoadActFuncSet(
                name=nc.get_next_instruction_name(),
                act_func_set_id=set_id, ins=[], outs=[])
            return nc.scalar.add_instruction(ins)

        exp_state = {"first": None, "last": None}

        HEAD_PACKS, QC_STRIPS = _build_head_packs()

        def scores_exp_qc(j, qc, gate_sins):
            """Scores + exp + causal mask for the packs whose LAST strip is
            in q chunk qc (dense cross-chunk packing).  gate_sins: sin
            instructions the first exp must follow (ACT-table grouping).
            Returns the head's ext-tile dict, filled incrementally."""
            if qc == 0:
                exp_state[("exts", j)] = {}
            exts = exp_state[("exts", j)]
            for pi_, (width, pack) in enumerate(HEAD_PACKS):
                if pack[-1][0] != qc or pi_ in exts:
                    continue
                sc = psp.tile([128, PACKW], F32, tag="ps")
                for (sqc, kb, qs, N, off) in pack:
                    nc.tensor.matmul(sc[:, off:off + N],
                                     QKT[j][:, 1, 128 * kb:128 * kb + 128],
                                     QKT[j][:, 0, qs:qs + N],
                                     start=True, stop=True)
                ext = expool.tile([128, PACKW], F16, tag="ex")
                exts[pi_] = ext
                e = nc.scalar.activation(out=ext[:, 0:width],
                                         in_=sc[:, 0:width],
                                         func=AF.Exp, bias=bz[:, 0:1], scale=EXP_SCALE)
                for si in gate_sins:
                    bass._add_dep_helper(e.ins, si.ins, sync=True,
                                         reason="act-table-order")
                gate_sins = []
                exp_state["last"] = e
                for (sqc, kb, qs, N, off) in pack:
                    if kb >= 4 * sqc:  # diagonal strip: zero exp where q < k
                        nc.gpsimd.affine_select(
                            out=ext[:, off:off + 128], in_=ext[:, off:off + 128],
                            pattern=[[1, 128]], compare_op=ALU.is_ge, fill=0.0,
                            base=0, channel_multiplier=-1)
            return exts

        def attnv_qc(j, qc, exts):
            """attn@V accumulation + renormalization for one q chunk."""
            ot_ps = pso.tile([65, 512], F32, tag="po")
            n_av = 4 * qc + 4
            avi = 0
            for (pi_, kb, qs, N, off) in QC_STRIPS[qc]:
                q0 = qs - 512 * qc
                nc.tensor.matmul(ot_ps[:, q0:q0 + N],
                                 vaug[j][:, kb, :],
                                 exts[pi_][:, off:off + N],
                                 start=(avi == 0), stop=(avi == n_av - 1))
                avi += 1
            ot_s = otpool.tile([65, 512], F32, tag="ot")
            nc.vector.tensor_copy(out=ot_s, in_=ot_ps)
            on_ps = psn.tile([128, 4, DH + 1], F32, tag="pn")
            for t4 in range(4):
                nc.tensor.transpose(on_ps[:, t4, :], ot_s[:, 128 * t4:128 * t4 + 128],
                                    identr[0:65, 0:65])
            rec = tiny.tile([128, 4], F32, tag="tiny")
            nc.vector.reciprocal(out=rec, in_=on_ps[:, :, DH:DH + 1])
            for t4 in range(4):
                nc.vector.scalar_tensor_tensor(
                    out=ro_pre[:, 4 * qc + t4, DH * j:DH * j + DH],
                    in0=on_ps[:, t4, 0:DH], scalar=rec[:, t4:t4 + 1],
                    in1=opr[:, DH * j:DH * j + DH],
                    op0=ALU.mult, op1=ALU.mult)

        out_r = out_d[:, :].rearrange("(n p) d -> p n d", p=128)
        fo_tiles = {}

        def out_pre(qq, last=False):
            """Final euler layer range reduction for s-quarter qq (DVE/Pool
            only; the sin runs after the last exp).  The last quarter's
            subtract runs on DVE - it sits on the kernel's tail path and
            DVE is idle there while Pool's TT is slow."""
            fo = mid.tile([128, 4, DC], F32, tag="mid", name=f"fo{qq}")
            io = midi.tile([128, 4, DC], I32, tag="midi", name=f"io{qq}")
            nc.vector.tensor_scalar(out=io, in0=ro_pre[:, 4 * qq:4 * qq + 4, :],
                                    scalar1=c_o, scalar2=None, op0=ALU.add)
            if last:
                nc.vector.scalar_tensor_tensor(
                    out=fo, in0=io, scalar=-1.0,
                    in1=ro_pre[:, 4 * qq:4 * qq + 4, :],
                    op0=ALU.mult, op1=ALU.add)
            else:
                nc.gpsimd.tensor_tensor(out=fo, in0=ro_pre[:, 4 * qq:4 * qq + 4, :],
                                        in1=io, op=ALU.subtract)
            fo_tiles[qq] = fo

        def out_fin(qq, split=1):
            fo = fo_tiles[qq]
            for h in range(split):
                nb = 4 // split
                fs = nc.scalar.activation(out=fo[:, h * nb:(h + 1) * nb, :],
                                          in_=fo[:, h * nb:(h + 1) * nb, :],
                                          func=AF.Sin, bias=bco[:, 0:1],
                                          scale=TWO_PI)
                bass._add_dep_helper(fs.ins, exp_state["last"].ins, sync=True,
                                     reason="act-table-order")
                nc.vector.tensor_scalar(out=fo[:, h * nb:(h + 1) * nb, :],
                                        in0=fo[:, h * nb:(h + 1) * nb, :],
                                        scalar1=SQRT2, scalar2=None, op0=ALU.mult)
                nc.sync.dma_start(
                    out=out_r[:, 4 * qq + h * nb:4 * qq + (h + 1) * nb, :],
                    in_=fo[:, h * nb:(h + 1) * nb, :])

        # ---------------- program ----------------
        # Per-head phase pipeline, fine-grained emission: engines execute
        # their queues in program order, so prep chunks of head j+1 are
        # emitted BETWEEN attention chunks of head j (ditto final-layer
        # quarters during head 3).  ACT-table grouping (sins j+1 after exps
        # j) is enforced with sync deps.
        for _q in range(4):
            prep_cc(0, _q)
        qk_sin(0)
        for _q in range(4):
            v_quarter(_q)
        for j in range(NH):
            gates = list(sin_insts) if j == 0 else []
            pe_prev = None
            for qc in range(4):
                if j + 1 < NH:
                    prep_cc(j + 1, qc)
                pe_cur = scores_exp_qc(j, qc, gates if qc == 0 else [])
                if pe_prev is not None:
                    attnv_qc(j, qc - 1, pe_prev)
                    if j + 1 == NH:
                        out_pre(qc - 1)
                pe_prev = pe_cur
            attnv_qc(j, 3, pe_prev)
            if j + 1 < NH:
                sin_j = qk_sin(j + 1)
                bass._add_dep_helper(sin_j.ins, exp_state["last"].ins,
                                     sync=True, reason="act-table-order")
                pl = preload_table(EXP_SET)
                bass._add_dep_helper(pl.ins, sin_j.ins, sync=True,
                                     reason="act-table-order")
        out_pre(3, last=True)
        pl = preload_table(SIN_SET)
        bass._add_dep_helper(pl.ins, exp_state["last"].ins, sync=True,
                             reason="act-table-order")
        for qq in range(3):
            out_fin(qq)
        out_fin(3, split=2)

    nc.finalize()
    return nc


def _host_params(inputs, c):
    """Per-core input dict for core c."""
    b, g = c // 4, c % 4
    inv2pi = 1.0 / (2.0 * np.pi)
    x = np.asarray(inputs["x"], dtype=np.float32)
    xin = np.ascontiguousarray(x[b, :, DC * g:DC * g + DC])

    def f64(a):
        return np.asarray(a, dtype=np.float64)

    qkp = np.zeros((128, NH, 6), dtype=np.float32)
    rows = np.arange(128) % DH
    cos_row = (np.arange(128) < 64).astype(np.float64) * 0.25
    for j in range(NH):
        h = NH * g + j
        for pi, (wn, bn, pn) in enumerate([("w_q", "b_q", "phi_q"),
                                           ("w_k", "b_k", "phi_k")]):
            w = f64(inputs[wn])[h]
            bb = f64(inputs[bn])[h]
            ph = f64(inputs[pn])[h]
            qkp[:, j, 3 * pi + 0] = (inv2pi / (1.0 + np.abs(w)))[rows]
            qkp[:, j, 3 * pi + 1] = (ph * inv2pi)[rows]
            qkp[:, j, 3 * pi + 2] = (bb * inv2pi)[rows] + cos_row

    vp = np.zeros((128, DC), dtype=np.float32)
    wv = f64(inputs["w_v"])[NH * g:NH * g + NH].reshape(-1)
    vp[:, :] = (inv2pi / (1.0 + np.abs(wv)))[None, :]

    # final stage: ro_pre = on * rec * op with op = sqrt(2)/(2pi(1+|w_o|))
    op = np.zeros((128, DC), dtype=np.float32)
    wo = f64(inputs["w_out"])[DC * g:DC * g + DC]
    op[:, :] = (SQRT2 * inv2pi / (1.0 + np.abs(wo)))[None, :]

    pars = np.concatenate([qkp.reshape(128, NH * 6), vp, op,
                           np.eye(128, dtype=np.float32)], axis=1)
    pars = np.ascontiguousarray(pars.astype(np.float32))
    return {"xin": xin, "pars": pars}


_NC_CACHE = {}


def _tphi_signature(pars):
    qkp = pars[:, :NH * 6].reshape(128, NH, 6)
    cols = []
    for j in range(NH):
        for pi in range(2):
            cols.append(np.ascontiguousarray(
                qkp[:, j, (3 * pi + 1, 3 * pi + 2)]).tobytes())
    uniq = {}
    return tuple(uniq.setdefault(c, len(uniq)) for c in cols)


def kernel(**inputs) -> np.ndarray:
    in_maps = [_host_params(inputs, c) for c in range(8)]
    sigs = {_tphi_signature(m["pars"]) for m in in_maps}
    sig = sigs.pop() if len(sigs) == 1 else tuple(range(2 * NH))
    inv2pi = 1.0 / (2.0 * np.pi)
    bv = np.asarray(inputs["b_v"], dtype=np.float64).reshape(-1)
    bo = np.asarray(inputs["b_out"], dtype=np.float64).reshape(-1)
    assert np.all(bv == bv[0]) and np.all(bo == bo[0]), "non-uniform b_v/b_out unsupported"
    c_v = float(np.float32(bv[0] * inv2pi + 0.125))
    c_o = float(np.float32(bo[0] * inv2pi + 0.125))
    key = (sig, c_v, c_o)
    if _NC_CACHE.get("key") != key:
        _NC_CACHE["nc"] = build_nc(sig, c_v, c_o)
        _NC_CACHE["key"] = key
    nc = _NC_CACHE["nc"]
    res = run_bass_kernel_spmd(nc, in_maps, core_ids=list(range(8)))
    full = np.empty((B, S, D), dtype=np.float32)
    for c in range(8):
        b, g = c // 4, c % 4
        full[b, :, DC * g:DC * g + DC] = res.results[c]["out"]
    return full

